# revision 35
# baseline (speedup 1.0000x reference)
"""Multi-head causal attention (B=2, T=4096, H=8, D=64) on 8 TRN2 NeuronCores.

Sharding: core c handles batch b = c//4 and heads (2*(c%4), 2*(c%4)+1).

Architecture (vs the f32r baseline): inputs stream in bf16 via merged
row-interleaved DMAs (tile[p,i,c] = dram[4p+i,c] for both x and the
weights, so contraction chunks stay consistent). Attention S^T blocks are
packed COMPACTLY (diagonal chunks keep only their causal-valid q-suffix,
ordered p0,p1,p3,p2 so no matmul output crosses a PSUM bank boundary), so
the ACT-engine exp — the bottleneck engine — covers the minimal column
count. PV runs in a [q-partition, d-free] layout (N=65 accumulating
matmuls, ~half the PE cost of the [65, q] layout); the po accumulators are
DVE-memset per qi and accumulated with start=False only, because a PSUM
start=True pending-zeroes its whole 2KB bank (which would wipe sibling
subblock accumulations). Denominators ride along as a ones-column of vaug;
the normalized [q, d] output is transposed to [d, q] for the projection by
the XBAR transpose DMA. A PE warmup ramps the clock to 2.4GHz during the
input-DMA window, and qk/v/proj work is interleaved between S groups via a
pending queue so the exp stream never starves.

PSUM budget (8 banks): S^T slots 3+2, po accumulators 2, qkv/proj acc 1.
"""

import os
import sys

for _p in ("/opt/trn_rl_repo", "/root/.axon_site/_ro/trn_rl_repo"):
    if os.path.isdir(_p) and _p not in sys.path:
        sys.path.insert(0, _p)
        break

from contextlib import ExitStack

import ml_dtypes
import numpy as np

B, T, H, D = 2, 4096, 8, 64
C = H * D  # 512
NQT = T // 512  # 8 q-tiles of 512 queries
NKC = T // 128  # 32 k-chunks of 128 keys

MODE = os.environ.get("ATTN_MODE", "fast")

_cache = {}

CAPS = (1536, 1024)  # psS slot capacities (cols)


def _make_groups():
    """Per (qi, h) pack causal k-chunks into alternating PSUM slots.

    Chunk widths are compact: non-diag 512; diag chunk p keeps only its
    valid q-suffix of 512-128p cols. Returns list of groups with fields:
    qi, h, slot, W, chunks=[(c, off, w, p)].
    """
    groups = []
    slot = 0
    for qi in range(NQT):
        for h in range(2):
            chunks = [(c, 512, -1) for c in range(4 * qi)]
            # diag order p0,p1,p3,p2 packs widths 512/384/128/256 so no
            # matmul output crosses a 512-col PSUM bank boundary
            chunks += [(4 * qi + p, 512 - 128 * p, p) for p in (0, 1, 3, 2)]
            cur, off = [], 0
            for c, w, p in chunks:
                if (off % 512) + w > 512:
                    off = (off + 511) // 512 * 512  # bank-align (pad)
                if off + w > CAPS[slot]:
                    groups.append(dict(qi=qi, h=h, slot=slot, W=off,
                                       chunks=cur))
                    slot ^= 1
                    cur, off = [], 0
                cur.append((c, off, w, p))
                off += w
            if cur:
                groups.append(dict(qi=qi, h=h, slot=slot, W=off, chunks=cur))
                slot ^= 1
    return groups


def _build(has_bias=True):
    import concourse.mybir as mybir
    import concourse.tile as tile
    from concourse import bacc

    f32 = mybir.dt.float32
    f32r = mybir.dt.float32r
    bf16 = mybir.dt.bfloat16
    pdt = bf16 if MODE == "fast" else f32
    Exp = mybir.ActivationFunctionType.Exp

    nc = bacc.Bacc("TRN2", target_bir_lowering=False, debug=False,
                   enable_asserts=False)

    xt_d = nc.dram_tensor("xt", [C, T], bf16, kind="ExternalInput").ap()
    wqk_d = nc.dram_tensor("wqk", [C, 256], bf16, kind="ExternalInput").ap()
    wv_d = nc.dram_tensor("wv", [C, 128], bf16, kind="ExternalInput").ap()
    bqk_d = nc.dram_tensor("bqk", [128, 2], f32, kind="ExternalInput").ap()
    bv_d = nc.dram_tensor("bv", [128, 1], f32, kind="ExternalInput").ap()
    wp_d = nc.dram_tensor("wp", [128, C], bf16, kind="ExternalInput").ap()
    mask_d = nc.dram_tensor("mask", [128, 512], pdt,
                            kind="ExternalInput").ap()
    idn_d = nc.dram_tensor("idn", [128, 128], bf16,
                           kind="ExternalInput").ap()
    out_d = nc.dram_tensor("partial", [T, C], bf16,
                           kind="ExternalOutput").ap()
    dbg = os.environ.get("ATTN_DEBUG") == "1"
    if dbg:
        dbg_qt = nc.dram_tensor("dbg_qt", [128, T], bf16,
                                kind="ExternalOutput").ap()
        dbg_kt = nc.dram_tensor("dbg_kt", [128, T], bf16,
                                kind="ExternalOutput").ap()
        dbg_va = nc.dram_tensor("dbg_va", [128, 2 * NKC * 65], pdt,
                                kind="ExternalOutput").ap()
        dbg_ot = nc.dram_tensor("dbg_ot", [128, T], bf16,
                                kind="ExternalOutput").ap()
        dbg_p0 = nc.dram_tensor("dbg_p0", [128, 1536], bf16,
                                kind="ExternalOutput").ap()
        dbg_ou = nc.dram_tensor("dbg_ou", [128, 4 * 130], f32,
                                kind="ExternalOutput").ap()
        dbg_on = nc.dram_tensor("dbg_on", [128, T], bf16,
                                kind="ExternalOutput").ap()

    groups = _make_groups()

    with tile.TileContext(nc, trace_sim=False) as tc, ExitStack() as ctx:
        cp = ctx.enter_context(tc.tile_pool(name="const", bufs=1))
        acc = ctx.enter_context(tc.tile_pool(name="acc", bufs=1,
                                             space="PSUM"))
        pop = ctx.enter_context(tc.tile_pool(name="po", bufs=1,
                                             space="PSUM"))
        sp = ctx.enter_context(tc.tile_pool(name="spsum", bufs=1,
                                            space="PSUM"))
        pp = ctx.enter_context(tc.tile_pool(name="pbuf", bufs=4))
        wk = ctx.enter_context(tc.tile_pool(name="wrk", bufs=2))

        def const(shape, dt, tag):
            return cp.tile(shape, dt, tag=tag, name=tag)

        # merged-DMA layout: [128, 4, *] with tile[p, i, c] = dram[4p+i, c].
        # x and the weights use the SAME row interleave, so contraction
        # chunk i consistently covers input rows {4p+i} on both sides.
        xtall = const([128, 4, T], bf16, "xtall")
        wqkall = const([128, 4, 256], bf16, "wqkall")
        wvall = const([128, 4, 128], bf16, "wvall")
        bqk = const([128, 2], f32, "bqk")
        bv = const([128, 1], f32, "bv")
        ones1 = const([1, 128], bf16, "ones1")
        bvr = const([1, 128], bf16, "bvr")
        wpf = const([128, C], bf16, "wpf")
        mask = const([128, 512], pdt, "mask")
        idn = const([128, 128], bf16, "idn")
        qT = const([128, T], bf16, "qT")
        kT = const([128, T], bf16, "kT")
        # single h-interleaved V tensor: slot (2c+h)*65 holds chunk c, head h
        # (64 dims + ones column) -> one strided DVE copy fills both heads
        vaugall = const([128, 2 * NKC * 65], pdt, "vaugall")

        def vg(h, c):
            return vaugall[:, (2 * c + h) * 65:(2 * c + h + 1) * 65]
        oTS = const([128, T], bf16, "oTS")
        # one oN slice per (qi, subblock): the XBAR-transpose DMA reads oN
        # asynchronously, so slices are never reused
        oNall = const([128, T], bf16, "oNall")
        if dbg:
            dbgp = const([128, 1536], bf16, "dbgp")
            dbgu = const([128, 4 * 130], f32, "dbgu")
            nc.vector.memset(dbgp[:], 0.0)
            nc.vector.memset(dbgu[:], 0.0)

        # po bank tiles: subblock s -> po[s//2], cols (s%2)*130 : +130
        # (cols h*65 .. h*65+64 = dims, col h*65+64 = softmax denominator).
        # A fresh tile VERSION is taken per qi (emit_f) so the framework
        # orders the next qi's accumulation against this qi's tail reads.
        # No memset: the first PV matmul into each bank per qi carries
        # start=True, whose pending-zero clears the whole 2KB bank.
        po = [None, None]
        po_qi = [-1]
        po_started = [False, False]

        def po_version(qi):
            if po_qi[0] != qi:
                po_qi[0] = qi
                po[0] = pop.tile([128, 512], f32, tag="po0",
                                 name=f"po0_{qi}")
                po[1] = pop.tile([128, 512], f32, tag="po1",
                                 name=f"po1_{qi}")
                po_started[0] = po_started[1] = False

        # DMA order = critical-path-first: qk(tile0) inputs, mask/idn, xt
        # tiles 1-2 (gate qi=1/2 S groups), then V/proj-side tensors. Later
        # xt chunks are merged (fewer HWDGE issues, which serialize at
        # ~625ns apiece) and issued mid-loop.
        def xt_load(c0, c1):
            nc.sync.dma_start(xtall[:, :, c0 * 512:c1 * 512],
                              xt_d[:, c0 * 512:c1 * 512])

        # wqk on SP, xt0 + small consts on the ACT HWDGE queue (ACT is idle
        # for the first ~6us): the two HWDGE queues overlap DGE setup so the
        # critical qk inputs land ~1.2us sooner
        nc.sync.dma_start(wqkall[:], wqk_d[:])
        nc.scalar.dma_start(xtall[:, :, 0:512], xt_d[:, 0:512])
        if has_bias:
            nc.sync.dma_start(bqk[:], bqk_d[:])
            nc.sync.dma_start(bv[:], bv_d[:])
        nc.scalar.dma_start(mask[:], mask_d[:])
        nc.scalar.dma_start(idn[:], idn_d[:])
        xt_load(1, 3)
        nc.sync.dma_start(wvall[:], wv_d[:])
        nc.sync.dma_start(wpf[:], wp_d[:])

        # PE warmup: dummy matmuls so the PE p-state ramp reaches full
        # clock (2.4GHz) before the first real matmul; overlaps the input
        # DMA window. Reads a small memset tile (finite), writes po[1]
        # whose first real use (PV of qi=0) is far later.
        wup = cp.tile([128, 512], bf16, tag="wup", name="wup")
        nc.vector.memset(wup[:], 1.0)
        wups = acc.tile([128, 512], f32, tag="acc", name="wups")
        # fine-grained warmup keeps PE continuously busy through the input-DMA
        # window: the p-state ramp (-> 2.4GHz after 3us) resets whenever PE
        # idles, so without this the first qk matmuls run at 1.2GHz
        for _ in range(34):
            nc.tensor.matmul(wups[:, 0:128], lhsT=wup[:, 0:128],
                             rhs=wup[:, 0:128], start=True, stop=True)
        # vaug ones-columns init on GPSIMD so DVE stays clear for the
        # startup qk copies
        nc.gpsimd.memset(vaugall[:], 1.0)

        def emit_qk_mm(g, t, st, ps):
            for ci in (st, st + 1):
                nc.tensor.matmul(
                    ps,
                    lhsT=wqkall[:, ci, g * 128:(g + 1) * 128],
                    rhs=xtall[:, ci, t * 512:(t + 1) * 512],
                    start=(ci == 0), stop=(ci == 3))

        def emit_qk_cp(g, t, ps):
            dst = qT if g == 0 else kT
            dcol = dst[:, t * 512:(t + 1) * 512]
            if has_bias:
                if (t == 0 and g == 1) or t in (1, 2):
                    nc.scalar.add(dcol, ps, bqk[:, g:g + 1])
                else:
                    nc.vector.tensor_scalar_add(dcol, ps, bqk[:, g:g + 1])
            elif (t == 0 and g == 1) or t in (1, 2):
                # ACT is idle (or starved anyway) through tiles 0-1; these
                # copies there relieve the early DVE choke. t=0 q copy stays
                # on DVE so the two t=0 copies run in parallel.
                nc.scalar.copy(dcol, ps)
            else:
                nc.vector.tensor_copy(dcol, ps)

        def emit_vtr(t, cc, ps):
            # [tpos, vdim] V for one 128-key chunk; x is already bf16 so
            # the lhsT comes straight from xtall
            c = 4 * t + cc
            o = cc * 128
            for ci in range(4):
                nc.tensor.matmul(
                    ps, lhsT=xtall[:, ci, t * 512 + o:t * 512 + o + 128],
                    rhs=wvall[:, ci, :],
                    start=(ci == 0), stop=(not has_bias and ci == 3))
            if has_bias:
                nc.tensor.matmul(ps, lhsT=ones1[:], rhs=bvr[:],
                                 start=False, stop=True)
            nc.vector.tensor_copy(
                vaugall[:, 2 * c * 65:(2 * c + 2) * 65]
                .rearrange("p (b x) -> p b x", b=2)[:, :, 0:64],
                ps[:, 0:128].rearrange("p (b x) -> p b x", b=2))

        def emit_proj(ti, alt=-1, eng=None):
            if alt < 0:
                psP = acc.tile([128, 512], f32, tag="acc", name=f"pj{ti}")
                dst = psP[:]
            else:
                psP = sp.tile([128, CAPS[alt]], f32, tag=f"s{alt}",
                              name=f"pj{ti}")
                dst = psP[:, 0:512]
            nc.tensor.matmul(dst, lhsT=oTS[:, ti * 128:(ti + 1) * 128],
                             rhs=wpf[:], start=True, stop=True)
            ob = wk.tile([128, 512], bf16, tag="ob", bufs=8, name=f"ob{ti}")
            if eng == "s":
                nc.scalar.copy(ob[:], dst)
            else:
                nc.vector.tensor_copy(ob[:], dst)
            nc.sync.dma_start(out_d[ti * 128:(ti + 1) * 128, :], ob[:])

        def emit_s(g):
            qi, h = g["qi"], g["h"]
            hb = h * 64
            g["psS"] = sp.tile([128, CAPS[g["slot"]]], f32,
                               tag=f"s{g['slot']}",
                               name=f"s_{h}_{qi}_{g['chunks'][0][0]}")
            # high priority: S matmuls feed the bottleneck ACT exp stream,
            # so they must preempt PV/qk backlog in the PE scheduler heap
            with tc.high_priority():
                for c, off, w, p in g["chunks"]:
                    nc.tensor.matmul(
                        g["psS"][:, off:off + w],
                        lhsT=kT[hb:hb + 64, c * 128:(c + 1) * 128],
                        rhs=qT[hb:hb + 64,
                               qi * 512 + (512 - w):(qi + 1) * 512],
                        start=True, stop=True)

        def tail(qi, X, only_s=None):
            # DVE prep (normalize) runs inline; the PE transpose + oTS copy
            # + projection are deferred for qi<7 so they don't sit in the
            # in-order PE stream ahead of the next tile's qk/S matmuls.

            subs = (2 * X, 2 * X + 1) if only_s is None else (only_s,)
            for s in subs:
                base = (s % 2) * 130
                rr = wk.tile([128, 2], f32, tag="rr", bufs=4,
                             name=f"rr{qi}_{s}")
                oNs = oNall[:, (4 * qi + s) * 128:(4 * qi + s + 1) * 128]
                oUs = wk.tile([128, 130], f32, tag="oU", bufs=5,
                              name=f"oU{qi}_{s}")
                nc.vector.tensor_copy(oUs[:], po[X][:, base:base + 130])
                nc.vector.reciprocal_approx_fast(rr[:, 0:1],
                                                 oUs[:, 64:65])
                nc.vector.reciprocal_approx_fast(rr[:, 1:2],
                                                 oUs[:, 129:130])
                if dbg and qi == 1:
                    nc.vector.tensor_copy(dbgu[:, s * 130:(s + 1) * 130],
                                          oUs[:])
                nc.vector.tensor_scalar_mul(oNs[:, 0:64], oUs[:, 0:64],
                                            rr[:, 0:1])
                nc.vector.tensor_scalar_mul(oNs[:, 64:128],
                                            oUs[:, 65:129],
                                            rr[:, 1:2])
                ti = 4 * qi + s

                def fin(s=s, oNs=oNs, ti=ti):
                    # PE transpose (53ns) + Pool copy instead of the XBAR
                    # transpose DMA: each XBAR waits ~2.6us at the SP
                    # sequencer head, and that latency fed straight into the
                    # acc-bank version chain via the projection matmuls
                    psT = acc.tile([128, 1024], bf16, tag="acc",
                                   name=f"tr{ti}")
                    nc.tensor.transpose(psT[:, 0:128], oNs, idn[:])
                    nc.vector.tensor_copy(
                        oTS[:, qi * 512 + s * 128:qi * 512 + (s + 1) * 128],
                        psT[:, 0:128])

                if qi == NQT - 1:
                    fin()
                    # all final ob copies on ACT: it idles after the last
                    # exp while DVE still runs the s2/s3 tail chain
                    pending.append(("any", lambda ti=ti, a=(ti % 2):
                                    emit_proj(ti, a, "s")))
                else:
                    pending.append(("any", lambda ti=ti, f=fin:
                                    (f(), emit_proj(ti))))

        def emit_f(g):
            qi, h = g["qi"], g["h"]
            po_version(qi)
            last = (qi == NQT - 1 and h == 1
                    and any(p >= 0 for _, _, _, p in g["chunks"]))
            P = pp.tile([128, 1536], pdt, tag="p", bufs=9,
                        name=f"p_{h}_{qi}_{g['chunks'][0][0]}")

            def expmask(chunks):
                o0 = chunks[0][1]
                o1 = chunks[-1][1] + chunks[-1][2]
                nc.scalar.activation(P[:, o0:o1], g["psS"][:, o0:o1], Exp)
                for c, off, w, p in chunks:
                    if p >= 0:
                        nc.vector.tensor_mul(P[:, off:off + w],
                                             P[:, off:off + w],
                                             mask[:, 0:w])

            def pv(chunks):
                # PSUM start=True pending-zeroes the WHOLE bank, so each po
                # bank gets exactly ONE start (its first matmul of the qi);
                # all other matmuls accumulate onto pending-zero bytes.
                for c, off, w, p in chunks:
                    smin = max(p, 0)
                    for s in range(smin, 4):
                        pc = off + (s - smin) * 128 if p >= 0 \
                            else off + s * 128
                        c0 = (s % 2) * 130 + h * 65
                        first = not po_started[s // 2]
                        po_started[s // 2] = True
                        nc.tensor.matmul(
                            po[s // 2][:, c0:c0 + 65],
                            lhsT=P[:, pc:pc + 128],
                            rhs=vg(h, c),
                            start=first, stop=False,
                            skip_group_check=True)

            if last:
                # final-qi h=1 groups: per-chunk exp->mask->PV (+ tail as
                # soon as its stop-chunk lands) to shrink the drain
                for ch in g["chunks"]:
                    expmask([ch])
                    pv([ch])
                    pch = ch[0] - 4 * qi
                    # drain each subblock as its sum completes: s0 at p0,
                    # s1 at p1; s2 AND s3 both complete at p2 (p2 is emitted
                    # last and contributes to both)
                    if pch == 0:
                        tail(qi, 0, only_s=0)
                    elif pch == 1:
                        tail(qi, 0, only_s=1)
                    elif pch == 2:
                        tail(qi, 1)
            else:
                expmask(g["chunks"])
                if dbg and qi == 0 and h == 0:
                    nc.vector.tensor_copy(dbgp[:, 0:g["W"]], P[:, 0:g["W"]])
                pv(g["chunks"])
                if h == 1:
                    cs = [c for c, _, _, _ in g["chunks"]]
                    if 4 * qi + 1 in cs:
                        tail(qi, 0)
                    if 4 * qi + 2 in cs:
                        tail(qi, 1)

        # ---- main emission with software pipelining ----
        # pending holds (kind, fn): kind "in" = input-chain work for tile t
        # (qk/v/v-transposes, MUST be emitted before qi=t's S groups reference
        # qT/kT/vaug); kind "any" = projections (order-free). Input items are
        # always queued before "any" items, so draining the FIFO front
        # suffices at a qi boundary.
        pending = []

        def emit_inputs(t):
            """Returns (qk_items, v_items) for tile t (None past the end)."""
            if t >= NQT:
                return None
            st = {}

            def qk_start(g, t=t):
                st[g] = acc.tile([128, 512], f32, tag="acc",
                                 name=f"qk{g}_{t}")[:]
                emit_qk_mm(g, t, 0, st[g])

            def vchunk(cc, t=t):
                ps = acc.tile([128, 128], f32, tag="acc",
                              name=f"v{t}_{cc}")[:, 0:128]
                emit_vtr(t, cc, ps)

            qk_items = [
                lambda: qk_start(0),
                lambda: (emit_qk_mm(0, t, 2, st[0]), emit_qk_cp(0, t, st[0])),
                lambda: qk_start(1),
                lambda: (emit_qk_mm(1, t, 2, st[1]), emit_qk_cp(1, t, st[1])),
            ]
            v_items = [
                lambda: (vchunk(0), vchunk(1)),
                lambda: (vchunk(2), vchunk(3)),
            ]
            return qk_items, v_items

        from collections import deque
        qk0, v0 = emit_inputs(0)
        for it in qk0:
            it()  # qk(0) inline; v(0) queued
        pending.append((0, v0[0]))
        pending.append((0, v0[1]))

        gi = 0
        prevq = deque()
        LOOKAHEAD = 4
        for t in range(NQT):
            # input-chain for tile t must precede its S groups
            if t:
                while pending and pending[0][0] != "any" \
                        and pending[0][0] <= t:
                    pending.pop(0)[1]()
            if t == 0:
                xt_load(3, 5)
            elif t == 2:
                xt_load(5, 8)
            nxt = emit_inputs(t + 1)
            ngroups = sum(1 for g in groups[gi:] if g["qi"] == t)
            done = 0
            while gi < len(groups) and groups[gi]["qi"] == t:
                g = groups[gi]
                emit_s(g)
                prevq.append(g)
                gi += 1
                done += 1
                if done == min(2, ngroups) and nxt is not None:
                    # qk(t+1) inline right after the first S groups: its
                    # acc-bank versions then precede the tail/proj chain,
                    # so next-tile qT/kT never wait on tail latency
                    for it in nxt[0]:
                        it()
                    for it in nxt[1]:
                        pending.append((t + 1, it))
                    nxt = None
                left = ngroups - done
                npop = 1 if left and len(pending) <= left else \
                    (2 if left and len(pending) <= 2 * left else 3)
                for _ in range(min(npop, len(pending))):
                    pending.pop(0)[1]()
                if len(prevq) > LOOKAHEAD:
                    emit_f(prevq.popleft())
        while prevq:
            emit_f(prevq.popleft())
        while pending:
            pending.pop(0)[1]()

        if dbg:
            nc.sync.dma_start(dbg_qt[:], qT[:])
            nc.sync.dma_start(dbg_kt[:], kT[:])
            nc.sync.dma_start(dbg_va[:], vaugall[:])
            nc.sync.dma_start(dbg_ot[:], oTS[:])
            nc.sync.dma_start(dbg_on[:], oNall[:])
            nc.sync.dma_start(dbg_p0[:], dbgp[:])
            nc.sync.dma_start(dbg_ou[:], dbgu[:])

    nc.compile()
    return nc


def _get_nc(has_bias=True):
    key = f"nc{has_bias}"
    if key not in _cache:
        _cache[key] = _build(has_bias)
    return _cache[key]


def _prep_inputs(x, w_qkv, b_qkv, w_proj):
    x = np.asarray(x, np.float32)
    w_qkv = np.asarray(w_qkv, np.float32)
    b_qkv = np.asarray(b_qkv, np.float32)
    bf = ml_dtypes.bfloat16
    pdt_np = bf if MODE == "fast" else np.float32

    # compact causal mask tile: mask[k, i] = (k <= i); diag chunk p uses
    # the first 512-128p cols
    k_idx = np.arange(128)[:, None]
    i_idx = np.arange(512)[None, :]
    mask = (k_idx <= i_idx).astype(pdt_np)
    idn = np.eye(128, dtype=bf)

    in_maps = []
    for c in range(8):
        b = c // 4
        h0 = 2 * (c % 4)
        cols = slice(h0 * 64, (h0 + 2) * 64)  # 128 contiguous dims (2 heads)
        xt = np.ascontiguousarray(x[b].T)
        wq = w_qkv[:, :C][:, cols] * 0.125
        wkk = w_qkv[:, C:2 * C][:, cols]
        wvv = w_qkv[:, 2 * C:][:, cols]
        bq = b_qkv[:C][cols] * 0.125
        bk = b_qkv[C:2 * C][cols]
        bvv = b_qkv[2 * C:][cols]
        in_maps.append({
            "xt": np.ascontiguousarray(xt.astype(bf)),
            "wqk": np.ascontiguousarray(np.concatenate([wq, wkk], axis=1).astype(bf)),
            "wv": np.ascontiguousarray(wvv.astype(bf)),
            "bqk": np.ascontiguousarray(np.stack([bq, bk], axis=1)),
            "bv": np.ascontiguousarray(bvv[:, None]),
            "wp": np.ascontiguousarray(
                np.asarray(w_proj, np.float32)[cols, :].astype(bf)),
            "mask": mask,
            "idn": idn,
        })
    return in_maps


def kernel(x, w_qkv, b_qkv, w_proj, b_proj, _want_trace=False):
    from concourse.bass_utils import run_bass_kernel_spmd

    has_bias = bool(np.any(np.asarray(b_qkv)))
    nc = _get_nc(has_bias)
    in_maps = _prep_inputs(x, w_qkv, b_qkv, w_proj)
    res = run_bass_kernel_spmd(nc, in_maps, list(range(8)),
                               trace=_want_trace)
    if _want_trace:
        _cache["last_result"] = res
    out = np.zeros((B, T, C), np.float32)
    for c in range(8):
        out[c // 4] += np.asarray(res.results[c]["partial"], np.float32)
    out += np.asarray(b_proj, np.float32)[None, None, :]
    return out



# revision 36
# speedup vs baseline: 1.0205x; 1.0205x over previous
"""Multi-head causal attention (B=2, T=4096, H=8, D=64) on 8 TRN2 NeuronCores.

Sharding: core c handles batch b = c//4 and heads (2*(c%4), 2*(c%4)+1).

Architecture (vs the f32r baseline): inputs stream in bf16 via merged
row-interleaved DMAs (tile[p,i,c] = dram[4p+i,c] for both x and the
weights, so contraction chunks stay consistent). Attention S^T blocks are
packed COMPACTLY (diagonal chunks keep only their causal-valid q-suffix,
ordered p0,p1,p3,p2 so no matmul output crosses a PSUM bank boundary), so
the ACT-engine exp — the bottleneck engine — covers the minimal column
count. PV runs in a [q-partition, d-free] layout (N=65 accumulating
matmuls, ~half the PE cost of the [65, q] layout); the po accumulators are
DVE-memset per qi and accumulated with start=False only, because a PSUM
start=True pending-zeroes its whole 2KB bank (which would wipe sibling
subblock accumulations). Denominators ride along as a ones-column of vaug;
the normalized [q, d] output is transposed to [d, q] for the projection by
the XBAR transpose DMA. A PE warmup ramps the clock to 2.4GHz during the
input-DMA window, and qk/v/proj work is interleaved between S groups via a
pending queue so the exp stream never starves.

PSUM budget (8 banks): S^T slots 3+2, po accumulators 2, qkv/proj acc 1.
"""

import os
import sys

for _p in ("/opt/trn_rl_repo", "/root/.axon_site/_ro/trn_rl_repo"):
    if os.path.isdir(_p) and _p not in sys.path:
        sys.path.insert(0, _p)
        break

from contextlib import ExitStack

import ml_dtypes
import numpy as np

B, T, H, D = 2, 4096, 8, 64
C = H * D  # 512
NQT = T // 512  # 8 q-tiles of 512 queries
NKC = T // 128  # 32 k-chunks of 128 keys

MODE = os.environ.get("ATTN_MODE", "fast")

_cache = {}

CAPS = (1536, 1024)  # psS slot capacities (cols)


def _make_groups():
    """Per (qi, h) pack causal k-chunks into alternating PSUM slots.

    Chunk widths are compact: non-diag 512; diag chunk p keeps only its
    valid q-suffix of 512-128p cols. Returns list of groups with fields:
    qi, h, slot, W, chunks=[(c, off, w, p)].
    """
    groups = []
    slot = 0
    for qi in range(NQT):
        for h in range(2):
            chunks = [(c, 512, -1) for c in range(4 * qi)]
            # diag order p0,p1,p3,p2 packs widths 512/384/128/256 so no
            # matmul output crosses a 512-col PSUM bank boundary
            chunks += [(4 * qi + p, 512 - 128 * p, p) for p in (0, 1, 3, 2)]
            cur, off = [], 0
            for c, w, p in chunks:
                if (off % 512) + w > 512:
                    off = (off + 511) // 512 * 512  # bank-align (pad)
                if off + w > CAPS[slot]:
                    groups.append(dict(qi=qi, h=h, slot=slot, W=off,
                                       chunks=cur))
                    slot ^= 1
                    cur, off = [], 0
                cur.append((c, off, w, p))
                off += w
            if cur:
                groups.append(dict(qi=qi, h=h, slot=slot, W=off, chunks=cur))
                slot ^= 1
    return groups


def _build(has_bias=True):
    import concourse.mybir as mybir
    import concourse.tile as tile
    from concourse import bacc

    f32 = mybir.dt.float32
    f32r = mybir.dt.float32r
    bf16 = mybir.dt.bfloat16
    pdt = bf16 if MODE == "fast" else f32
    Exp = mybir.ActivationFunctionType.Exp

    nc = bacc.Bacc("TRN2", target_bir_lowering=False, debug=False,
                   enable_asserts=False)

    xt_d = nc.dram_tensor("xt", [C, T], bf16, kind="ExternalInput").ap()
    wqk_d = nc.dram_tensor("wqk", [C, 256], bf16, kind="ExternalInput").ap()
    wv_d = nc.dram_tensor("wv", [C, 128], bf16, kind="ExternalInput").ap()
    bqk_d = nc.dram_tensor("bqk", [128, 2], f32, kind="ExternalInput").ap()
    bv_d = nc.dram_tensor("bv", [128, 1], f32, kind="ExternalInput").ap()
    wp_d = nc.dram_tensor("wp", [128, C], bf16, kind="ExternalInput").ap()
    mask_d = nc.dram_tensor("mask", [128, 512], pdt,
                            kind="ExternalInput").ap()
    idn_d = nc.dram_tensor("idn", [128, 128], bf16,
                           kind="ExternalInput").ap()
    out_d = nc.dram_tensor("partial", [T, C], bf16,
                           kind="ExternalOutput").ap()
    dbg = os.environ.get("ATTN_DEBUG") == "1"
    if dbg:
        dbg_qt = nc.dram_tensor("dbg_qt", [128, T], bf16,
                                kind="ExternalOutput").ap()
        dbg_kt = nc.dram_tensor("dbg_kt", [128, T], bf16,
                                kind="ExternalOutput").ap()
        dbg_va = nc.dram_tensor("dbg_va", [128, 2 * NKC * 65], pdt,
                                kind="ExternalOutput").ap()
        dbg_ot = nc.dram_tensor("dbg_ot", [128, T], bf16,
                                kind="ExternalOutput").ap()
        dbg_p0 = nc.dram_tensor("dbg_p0", [128, 1536], bf16,
                                kind="ExternalOutput").ap()
        dbg_ou = nc.dram_tensor("dbg_ou", [128, 4 * 130], f32,
                                kind="ExternalOutput").ap()
        dbg_on = nc.dram_tensor("dbg_on", [128, T], bf16,
                                kind="ExternalOutput").ap()

    groups = _make_groups()

    with tile.TileContext(nc, trace_sim=False) as tc, ExitStack() as ctx:
        cp = ctx.enter_context(tc.tile_pool(name="const", bufs=1))
        acc = ctx.enter_context(tc.tile_pool(name="acc", bufs=1,
                                             space="PSUM"))
        pop = ctx.enter_context(tc.tile_pool(name="po", bufs=1,
                                             space="PSUM"))
        sp = ctx.enter_context(tc.tile_pool(name="spsum", bufs=1,
                                            space="PSUM"))
        pp = ctx.enter_context(tc.tile_pool(name="pbuf", bufs=4))
        wk = ctx.enter_context(tc.tile_pool(name="wrk", bufs=2))

        def const(shape, dt, tag):
            return cp.tile(shape, dt, tag=tag, name=tag)

        # merged-DMA layout: [128, 4, *] with tile[p, i, c] = dram[4p+i, c].
        # x and the weights use the SAME row interleave, so contraction
        # chunk i consistently covers input rows {4p+i} on both sides.
        xtall = const([128, 4, T], bf16, "xtall")
        wqkall = const([128, 4, 256], bf16, "wqkall")
        wvall = const([128, 4, 128], bf16, "wvall")
        bqk = const([128, 2], f32, "bqk")
        bv = const([128, 1], f32, "bv")
        ones1 = const([1, 128], bf16, "ones1")
        bvr = const([1, 128], bf16, "bvr")
        wpf = const([128, C], bf16, "wpf")
        mask = const([128, 512], pdt, "mask")
        idn = const([128, 128], bf16, "idn")
        qT = const([128, T], bf16, "qT")
        kT = const([128, T], bf16, "kT")
        # single h-interleaved V tensor: slot (2c+h)*65 holds chunk c, head h
        # (64 dims + ones column) -> one strided DVE copy fills both heads
        vaugall = const([128, 2 * NKC * 65], pdt, "vaugall")

        def vg(h, c):
            return vaugall[:, (2 * c + h) * 65:(2 * c + h + 1) * 65]
        oTS = const([128, T], bf16, "oTS")
        # one oN slice per (qi, subblock): the XBAR-transpose DMA reads oN
        # asynchronously, so slices are never reused
        oNall = const([128, T], bf16, "oNall")
        if dbg:
            dbgp = const([128, 1536], bf16, "dbgp")
            dbgu = const([128, 4 * 130], f32, "dbgu")
            nc.vector.memset(dbgp[:], 0.0)
            nc.vector.memset(dbgu[:], 0.0)

        # po bank tiles: subblock s -> po[s//2], cols (s%2)*130 : +130
        # (cols h*65 .. h*65+64 = dims, col h*65+64 = softmax denominator).
        # A fresh tile VERSION is taken per qi (emit_f) so the framework
        # orders the next qi's accumulation against this qi's tail reads.
        # No memset: the first PV matmul into each bank per qi carries
        # start=True, whose pending-zero clears the whole 2KB bank.
        po = [None, None]
        po_qi = [-1]
        po_started = [False, False]

        def po_version(qi):
            if po_qi[0] != qi:
                po_qi[0] = qi
                po[0] = pop.tile([128, 512], f32, tag="po0",
                                 name=f"po0_{qi}")
                po[1] = pop.tile([128, 512], f32, tag="po1",
                                 name=f"po1_{qi}")
                po_started[0] = po_started[1] = False

        # DMA order = critical-path-first: qk(tile0) inputs, mask/idn, xt
        # tiles 1-2 (gate qi=1/2 S groups), then V/proj-side tensors. Later
        # xt chunks are merged (fewer HWDGE issues, which serialize at
        # ~625ns apiece) and issued mid-loop.
        def xt_load(c0, c1):
            nc.sync.dma_start(xtall[:, :, c0 * 512:c1 * 512],
                              xt_d[:, c0 * 512:c1 * 512])

        # wqk on SP, xt0 + small consts on the ACT HWDGE queue (ACT is idle
        # for the first ~6us): the two HWDGE queues overlap DGE setup so the
        # critical qk inputs land ~1.2us sooner
        nc.sync.dma_start(wqkall[:], wqk_d[:])
        nc.scalar.dma_start(xtall[:, :, 0:512], xt_d[:, 0:512])
        if has_bias:
            nc.sync.dma_start(bqk[:], bqk_d[:])
            nc.sync.dma_start(bv[:], bv_d[:])
        nc.scalar.dma_start(mask[:], mask_d[:])
        nc.scalar.dma_start(idn[:], idn_d[:])
        xt_load(1, 3)
        nc.sync.dma_start(wvall[:], wv_d[:])
        nc.sync.dma_start(wpf[:], wp_d[:])

        # PE warmup: dummy matmuls so the PE p-state ramp reaches full
        # clock (2.4GHz) before the first real matmul; overlaps the input
        # DMA window. Reads a small memset tile (finite), writes po[1]
        # whose first real use (PV of qi=0) is far later.
        wup = cp.tile([128, 512], bf16, tag="wup", name="wup")
        nc.vector.memset(wup[:], 1.0)
        wups = acc.tile([128, 512], f32, tag="acc", name="wups")
        # fine-grained warmup keeps PE continuously busy through the input-DMA
        # window: the p-state ramp (-> 2.4GHz after 3us) resets whenever PE
        # idles, so without this the first qk matmuls run at 1.2GHz
        for _ in range(34):
            nc.tensor.matmul(wups[:, 0:128], lhsT=wup[:, 0:128],
                             rhs=wup[:, 0:128], start=True, stop=True)
        # vaug ones-columns init on GPSIMD so DVE stays clear for the
        # startup qk copies
        nc.gpsimd.memset(vaugall[:], 1.0)

        def emit_qk_mm(g, t, st, ps):
            for ci in (st, st + 1):
                nc.tensor.matmul(
                    ps,
                    lhsT=wqkall[:, ci, g * 128:(g + 1) * 128],
                    rhs=xtall[:, ci, t * 512:(t + 1) * 512],
                    start=(ci == 0), stop=(ci == 3))

        def emit_qk_cp(g, t, ps):
            dst = qT if g == 0 else kT
            dcol = dst[:, t * 512:(t + 1) * 512]
            if has_bias:
                if (t == 0 and g == 1) or t in (1, 2):
                    nc.scalar.add(dcol, ps, bqk[:, g:g + 1])
                else:
                    nc.vector.tensor_scalar_add(dcol, ps, bqk[:, g:g + 1])
            elif (t == 0 and g == 1) or t in (1, 2):
                # ACT is idle (or starved anyway) through tiles 0-1; these
                # copies there relieve the early DVE choke. t=0 q copy stays
                # on DVE so the two t=0 copies run in parallel.
                nc.scalar.copy(dcol, ps)
            else:
                nc.vector.tensor_copy(dcol, ps)

        def emit_vtr(t, cc, ps):
            # [tpos, vdim] V for one 128-key chunk; x is already bf16 so
            # the lhsT comes straight from xtall
            c = 4 * t + cc
            o = cc * 128
            for ci in range(4):
                nc.tensor.matmul(
                    ps, lhsT=xtall[:, ci, t * 512 + o:t * 512 + o + 128],
                    rhs=wvall[:, ci, :],
                    start=(ci == 0), stop=(not has_bias and ci == 3))
            if has_bias:
                nc.tensor.matmul(ps, lhsT=ones1[:], rhs=bvr[:],
                                 start=False, stop=True)
            nc.vector.tensor_copy(
                vaugall[:, 2 * c * 65:(2 * c + 2) * 65]
                .rearrange("p (b x) -> p b x", b=2)[:, :, 0:64],
                ps[:, 0:128].rearrange("p (b x) -> p b x", b=2))

        def emit_proj(ti, alt=-1, eng=None):
            if alt < 0:
                psP = acc.tile([128, 512], f32, tag="acc", name=f"pj{ti}")
                dst = psP[:]
            else:
                psP = sp.tile([128, CAPS[alt]], f32, tag=f"s{alt}",
                              name=f"pj{ti}")
                dst = psP[:, 0:512]
            nc.tensor.matmul(dst, lhsT=oTS[:, ti * 128:(ti + 1) * 128],
                             rhs=wpf[:], start=True, stop=True)
            ob = wk.tile([128, 512], bf16, tag="ob", bufs=8, name=f"ob{ti}")
            if eng == "s":
                nc.scalar.copy(ob[:], dst)
            else:
                nc.vector.tensor_copy(ob[:], dst)
            nc.sync.dma_start(out_d[ti * 128:(ti + 1) * 128, :], ob[:])

        def emit_s(g):
            qi, h = g["qi"], g["h"]
            hb = h * 64
            g["psS"] = sp.tile([128, CAPS[g["slot"]]], f32,
                               tag=f"s{g['slot']}",
                               name=f"s_{h}_{qi}_{g['chunks'][0][0]}")
            for c, off, w, p in g["chunks"]:
                nc.tensor.matmul(
                    g["psS"][:, off:off + w],
                    lhsT=kT[hb:hb + 64, c * 128:(c + 1) * 128],
                    rhs=qT[hb:hb + 64,
                           qi * 512 + (512 - w):(qi + 1) * 512],
                    start=True, stop=True)

        def tail(qi, X, only_s=None):
            # DVE prep (normalize) runs inline; the PE transpose + oTS copy
            # + projection are deferred for qi<7 so they don't sit in the
            # in-order PE stream ahead of the next tile's qk/S matmuls.

            subs = (2 * X, 2 * X + 1) if only_s is None else (only_s,)
            for s in subs:
                base = (s % 2) * 130
                rr = wk.tile([128, 2], f32, tag="rr", bufs=4,
                             name=f"rr{qi}_{s}")
                oNs = oNall[:, (4 * qi + s) * 128:(4 * qi + s + 1) * 128]
                oUs = wk.tile([128, 130], f32, tag="oU", bufs=5,
                              name=f"oU{qi}_{s}")
                nc.vector.tensor_copy(oUs[:], po[X][:, base:base + 130])
                nc.vector.reciprocal_approx_fast(rr[:, 0:1],
                                                 oUs[:, 64:65])
                nc.vector.reciprocal_approx_fast(rr[:, 1:2],
                                                 oUs[:, 129:130])
                if dbg and qi == 1:
                    nc.vector.tensor_copy(dbgu[:, s * 130:(s + 1) * 130],
                                          oUs[:])
                nc.vector.tensor_scalar_mul(oNs[:, 0:64], oUs[:, 0:64],
                                            rr[:, 0:1])
                nc.vector.tensor_scalar_mul(oNs[:, 64:128],
                                            oUs[:, 65:129],
                                            rr[:, 1:2])
                ti = 4 * qi + s

                def fin(s=s, oNs=oNs, ti=ti):
                    # PE transpose (53ns) + Pool copy instead of the XBAR
                    # transpose DMA: each XBAR waits ~2.6us at the SP
                    # sequencer head, and that latency fed straight into the
                    # acc-bank version chain via the projection matmuls
                    psT = acc.tile([128, 1024], bf16, tag="acc",
                                   name=f"tr{ti}")
                    nc.tensor.transpose(psT[:, 0:128], oNs, idn[:])
                    nc.vector.tensor_copy(
                        oTS[:, qi * 512 + s * 128:qi * 512 + (s + 1) * 128],
                        psT[:, 0:128])

                if qi == NQT - 1:
                    fin()
                    # all final ob copies on ACT: it idles after the last
                    # exp while DVE still runs the s2/s3 tail chain
                    pending.append(("any", lambda ti=ti, a=(ti % 2):
                                    emit_proj(ti, a, "s")))
                else:
                    pending.append(("any", lambda ti=ti, f=fin:
                                    (f(), emit_proj(ti))))

        def emit_f(g):
            qi, h = g["qi"], g["h"]
            po_version(qi)
            last = (qi == NQT - 1 and h == 1
                    and any(p >= 0 for _, _, _, p in g["chunks"]))
            P = pp.tile([128, 1536], pdt, tag="p", bufs=9,
                        name=f"p_{h}_{qi}_{g['chunks'][0][0]}")

            def expmask(chunks):
                o0 = chunks[0][1]
                o1 = chunks[-1][1] + chunks[-1][2]
                nc.scalar.activation(P[:, o0:o1], g["psS"][:, o0:o1], Exp)
                for c, off, w, p in chunks:
                    if p >= 0:
                        nc.vector.tensor_mul(P[:, off:off + w],
                                             P[:, off:off + w],
                                             mask[:, 0:w])

            def pv(chunks):
                # PSUM start=True pending-zeroes the WHOLE bank, so each po
                # bank gets exactly ONE start (its first matmul of the qi);
                # all other matmuls accumulate onto pending-zero bytes.
                for c, off, w, p in chunks:
                    smin = max(p, 0)
                    for s in range(smin, 4):
                        pc = off + (s - smin) * 128 if p >= 0 \
                            else off + s * 128
                        c0 = (s % 2) * 130 + h * 65
                        first = not po_started[s // 2]
                        po_started[s // 2] = True
                        nc.tensor.matmul(
                            po[s // 2][:, c0:c0 + 65],
                            lhsT=P[:, pc:pc + 128],
                            rhs=vg(h, c),
                            start=first, stop=False,
                            skip_group_check=True)

            if last:
                # final-qi h=1 groups: per-chunk exp->mask->PV (+ tail as
                # soon as its stop-chunk lands) to shrink the drain
                for ch in g["chunks"]:
                    expmask([ch])
                    pv([ch])
                    pch = ch[0] - 4 * qi
                    # drain each subblock as its sum completes: s0 at p0,
                    # s1 at p1; s2 AND s3 both complete at p2 (p2 is emitted
                    # last and contributes to both)
                    if pch == 0:
                        tail(qi, 0, only_s=0)
                    elif pch == 1:
                        tail(qi, 0, only_s=1)
                    elif pch == 2:
                        tail(qi, 1)
            else:
                expmask(g["chunks"])
                if dbg and qi == 0 and h == 0:
                    nc.vector.tensor_copy(dbgp[:, 0:g["W"]], P[:, 0:g["W"]])
                pv(g["chunks"])
                if h == 1:
                    cs = [c for c, _, _, _ in g["chunks"]]
                    if 4 * qi + 1 in cs:
                        tail(qi, 0)
                    if 4 * qi + 2 in cs:
                        tail(qi, 1)

        # ---- main emission with software pipelining ----
        # pending holds (kind, fn): kind "in" = input-chain work for tile t
        # (qk/v/v-transposes, MUST be emitted before qi=t's S groups reference
        # qT/kT/vaug); kind "any" = projections (order-free). Input items are
        # always queued before "any" items, so draining the FIFO front
        # suffices at a qi boundary.
        pending = []

        def emit_inputs(t):
            """Returns (qk_items, v_items) for tile t (None past the end)."""
            if t >= NQT:
                return None
            st = {}

            def qk_start(g, t=t):
                st[g] = acc.tile([128, 512], f32, tag="acc",
                                 name=f"qk{g}_{t}")[:]
                emit_qk_mm(g, t, 0, st[g])

            def vchunk(cc, t=t):
                ps = acc.tile([128, 128], f32, tag="acc",
                              name=f"v{t}_{cc}")[:, 0:128]
                emit_vtr(t, cc, ps)

            qk_items = [
                lambda: qk_start(0),
                lambda: (emit_qk_mm(0, t, 2, st[0]), emit_qk_cp(0, t, st[0])),
                lambda: qk_start(1),
                lambda: (emit_qk_mm(1, t, 2, st[1]), emit_qk_cp(1, t, st[1])),
            ]
            v_items = [
                lambda: (vchunk(0), vchunk(1)),
                lambda: (vchunk(2), vchunk(3)),
            ]
            return qk_items, v_items

        from collections import deque
        qk0, v0 = emit_inputs(0)
        for it in qk0:
            it()  # qk(0) inline; v(0) queued
        pending.append((0, v0[0]))
        pending.append((0, v0[1]))

        gi = 0
        prevq = deque()
        LOOKAHEAD = 4
        for t in range(NQT):
            # input-chain for tile t must precede its S groups
            if t:
                while pending and pending[0][0] != "any" \
                        and pending[0][0] <= t:
                    pending.pop(0)[1]()
            if t == 0:
                xt_load(3, 5)
            elif t == 2:
                xt_load(5, 8)
            nxt = emit_inputs(t + 1)
            ngroups = sum(1 for g in groups[gi:] if g["qi"] == t)
            done = 0
            while gi < len(groups) and groups[gi]["qi"] == t:
                g = groups[gi]
                emit_s(g)
                prevq.append(g)
                gi += 1
                done += 1
                if done == min(2, ngroups) and nxt is not None:
                    # qk(t+1) inline right after the first S groups: its
                    # acc-bank versions then precede the tail/proj chain,
                    # so next-tile qT/kT never wait on tail latency
                    for it in nxt[0]:
                        it()
                    for it in nxt[1]:
                        pending.append((t + 1, it))
                    nxt = None
                left = ngroups - done
                npop = 1 if left and len(pending) <= left else \
                    (2 if left and len(pending) <= 2 * left else 3)
                for _ in range(min(npop, len(pending))):
                    pending.pop(0)[1]()
                if len(prevq) > LOOKAHEAD:
                    emit_f(prevq.popleft())
        while prevq:
            emit_f(prevq.popleft())
        while pending:
            pending.pop(0)[1]()

        if dbg:
            nc.sync.dma_start(dbg_qt[:], qT[:])
            nc.sync.dma_start(dbg_kt[:], kT[:])
            nc.sync.dma_start(dbg_va[:], vaugall[:])
            nc.sync.dma_start(dbg_ot[:], oTS[:])
            nc.sync.dma_start(dbg_on[:], oNall[:])
            nc.sync.dma_start(dbg_p0[:], dbgp[:])
            nc.sync.dma_start(dbg_ou[:], dbgu[:])

    nc.compile()
    return nc


def _get_nc(has_bias=True):
    key = f"nc{has_bias}"
    if key not in _cache:
        _cache[key] = _build(has_bias)
    return _cache[key]


def _prep_inputs(x, w_qkv, b_qkv, w_proj):
    x = np.asarray(x, np.float32)
    w_qkv = np.asarray(w_qkv, np.float32)
    b_qkv = np.asarray(b_qkv, np.float32)
    bf = ml_dtypes.bfloat16
    pdt_np = bf if MODE == "fast" else np.float32

    # compact causal mask tile: mask[k, i] = (k <= i); diag chunk p uses
    # the first 512-128p cols
    k_idx = np.arange(128)[:, None]
    i_idx = np.arange(512)[None, :]
    mask = (k_idx <= i_idx).astype(pdt_np)
    idn = np.eye(128, dtype=bf)

    in_maps = []
    for c in range(8):
        b = c // 4
        h0 = 2 * (c % 4)
        cols = slice(h0 * 64, (h0 + 2) * 64)  # 128 contiguous dims (2 heads)
        xt = np.ascontiguousarray(x[b].T)
        wq = w_qkv[:, :C][:, cols] * 0.125
        wkk = w_qkv[:, C:2 * C][:, cols]
        wvv = w_qkv[:, 2 * C:][:, cols]
        bq = b_qkv[:C][cols] * 0.125
        bk = b_qkv[C:2 * C][cols]
        bvv = b_qkv[2 * C:][cols]
        in_maps.append({
            "xt": np.ascontiguousarray(xt.astype(bf)),
            "wqk": np.ascontiguousarray(np.concatenate([wq, wkk], axis=1).astype(bf)),
            "wv": np.ascontiguousarray(wvv.astype(bf)),
            "bqk": np.ascontiguousarray(np.stack([bq, bk], axis=1)),
            "bv": np.ascontiguousarray(bvv[:, None]),
            "wp": np.ascontiguousarray(
                np.asarray(w_proj, np.float32)[cols, :].astype(bf)),
            "mask": mask,
            "idn": idn,
        })
    return in_maps


def kernel(x, w_qkv, b_qkv, w_proj, b_proj, _want_trace=False):
    from concourse.bass_utils import run_bass_kernel_spmd

    has_bias = bool(np.any(np.asarray(b_qkv)))
    nc = _get_nc(has_bias)
    in_maps = _prep_inputs(x, w_qkv, b_qkv, w_proj)
    res = run_bass_kernel_spmd(nc, in_maps, list(range(8)),
                               trace=_want_trace)
    if _want_trace:
        _cache["last_result"] = res
    out = np.zeros((B, T, C), np.float32)
    for c in range(8):
        out[c // 4] += np.asarray(res.results[c]["partial"], np.float32)
    out += np.asarray(b_proj, np.float32)[None, None, :]
    return out

